# revision 9
# baseline (speedup 1.0000x reference)
"""Trainium2 Bass kernel for a binarized transformer block (BiT-style).

Block (per batch element, forward only):
    h   = LN1(x);  s1 = sign(h)
    z   = s1 @ sign(w_qkv)^T          (alpha>0 dropped: only signs consumed)
    q,k,v = sign(z) split into heads  (+-1)
    S   = q @ k^T  (integer);  T = (S>0)   <- forward value of softmax-STE
    O   = T @ v    (integer);  so = sign(O)
    x1  = x + ls1*(so @ (a_p*sign(w_proj))^T + b_proj)
    h2  = LN2(x1)
    m   = gelu(h2 @ sign(w_fc1)^T * a1 + b1)
    out = x1 + ls2*(m @ (a2*sign(w_fc2))^T + b_fc2)

All binary matmuls are exact: +-1/{0,2} operands in fp8, fp32 PSUM
accumulation of integers.  Thresholds are Sign(2z+1) on odd integers, so
never evaluated at 0.  The proj/fc1/fc2 weights carry their per-channel
alpha scales (x64, fp8-rounded) folded in; the residual adds then use a
single scalar ls/64 scale when the layer-scale vector is uniform.
Sharding: batch 8 -> one element per NeuronCore, no collectives.
"""

import sys
import os

sys.path.insert(0, "/opt/trn_rl_repo")

import numpy as np
import ml_dtypes
from contextlib import ExitStack
from dataclasses import dataclass

from concourse import bass, bacc, mybir, tile
from concourse.masks import make_identity

P = 128
C = 768
CT = C // P          # 6 channel chunks
H = 12
HD = 64
HID = 3072
HT = HID // P        # 24 hidden chunks
OC = 3 * C           # 2304
B = 8
N_CORES = 8

F32 = mybir.dt.float32
BF16 = mybir.dt.bfloat16
FP8 = mybir.dt.float8e4
AF = mybir.ActivationFunctionType
AL = mybir.AluOpType

# heads whose S-binarize runs on ScalarE (+-1 encoding, colsum-corrected);
# the rest run on VectorE ({0,2} encoding, direct).
ACT_HEADS = frozenset(range(0, 12, 2))
DR = mybir.MatmulPerfMode.DoubleRow

# dev hook: CoreSim has no Gelu; dev_sim swaps this for Tanh on both sides.
GELU_FN = AF.Gelu


@dataclass(frozen=True)
class Cfg:
    nt: int = 8            # token tiles of 128 per core
    ln1_fast: bool = True  # ln1_b == 0 and ln1_g > 0 elementwise
    ln2_fast: bool = True  # ln2_g == 1 and ln2_b == 0
    has_cp2: bool = False  # ls1*b_proj != 0
    has_c2: bool = False   # ls2*b_fc2 != 0
    # uniform layer scales -> alpha-folded weights + scalar residual scale
    ls1s: float = 0.0      # ls1/64 when uniform, else 0 (tensor path)
    ls2s: float = 0.0


def _nchunks(n, step=512):
    out = []
    i = 0
    while i < n:
        out.append((i, min(step, n - i)))
        i += step
    return out


def build_program(cfg: Cfg, dbg=False):
    """Builds the per-core Bass program. Returns (nc, input_names)."""
    nt = cfg.nt
    N = nt * P
    NCH = _nchunks(N)
    uni1 = cfg.ls1s != 0.0
    uni2 = cfg.ls2s != 0.0

    dbg_t = {}

    def dbg_dump(nc, name, ap):
        if not dbg:
            return
        d = nc.dram_tensor(f"dbg_{name}", list(ap.shape), ap.dtype,
                           kind="ExternalOutput").ap()
        dbg_t[name] = d
        nc.sync.dma_start(d, ap)

    nc = bacc.Bacc("TRN2", target_bir_lowering=False, debug=False,
                   enable_asserts=False, num_devices=N_CORES)

    # ---- DRAM I/O -------------------------------------------------------
    x_d = nc.dram_tensor("x", [N, C], F32, kind="ExternalInput").ap()
    wqkvT_d = nc.dram_tensor("wqkvT", [C, OC], FP8, kind="ExternalInput").ap()
    wpT_d = nc.dram_tensor("wpT", [C, C], FP8, kind="ExternalInput").ap()
    w1T_d = nc.dram_tensor("w1T", [C, HID], FP8, kind="ExternalInput").ap()
    w2T_d = nc.dram_tensor("w2T", [HID, C], FP8, kind="ExternalInput").ap()
    b1s_d = nc.dram_tensor("b1s", [P, HT], F32, kind="ExternalInput").ap()
    opt_d = {}
    if not uni1:
        opt_d["cp1r"] = nc.dram_tensor("cp1r", [P, C], F32, kind="ExternalInput").ap()
    if not uni2:
        opt_d["c1r"] = nc.dram_tensor("c1r", [P, C], F32, kind="ExternalInput").ap()
    if cfg.has_cp2:
        opt_d["cp2r"] = nc.dram_tensor("cp2r", [P, C], F32, kind="ExternalInput").ap()
    if cfg.has_c2:
        opt_d["c2r"] = nc.dram_tensor("c2r", [P, C], F32, kind="ExternalInput").ap()
    if not cfg.ln1_fast:
        opt_d["g1r"] = nc.dram_tensor("g1r", [P, C], F32, kind="ExternalInput").ap()
        opt_d["b1r"] = nc.dram_tensor("b1r", [P, C], F32, kind="ExternalInput").ap()
    if not cfg.ln2_fast:
        opt_d["g2r"] = nc.dram_tensor("g2r", [P, C], F32, kind="ExternalInput").ap()
        opt_d["b2r"] = nc.dram_tensor("b2r", [P, C], F32, kind="ExternalInput").ap()
    out_d = nc.dram_tensor("out", [N, C], F32, kind="ExternalOutput").ap()

    with tile.TileContext(nc) as tc, ExitStack() as ctx:
        pc = ctx.enter_context(tc.tile_pool(name="const", bufs=1))
        px = ctx.enter_context(tc.tile_pool(name="xp", bufs=1))
        pwbig = ctx.enter_context(tc.tile_pool(name="wbig", bufs=2))
        pwp = ctx.enter_context(tc.tile_pool(name="wp", bufs=1))
        ptok = ctx.enter_context(tc.tile_pool(name="tok", bufs=1))
        pch = ctx.enter_context(tc.tile_pool(name="ch", bufs=2))
        pqk = ctx.enter_context(tc.tile_pool(name="qk", bufs=1))
        pv = ctx.enter_context(tc.tile_pool(name="vp", bufs=1))
        pst = ctx.enter_context(tc.tile_pool(name="st", bufs=4))
        pstat = ctx.enter_context(tc.tile_pool(name="stat", bufs=1))

        # PSUM: 3 rotating double-bank (4KB) slots shared by every
        # accumulator / S tile / transpose batch / warm-up (tag "S"), plus
        # one dedicated slot for the attention O accumulator.  3 slots let
        # the PE run a full step ahead of the two evacuation engines.
        pS = ctx.enter_context(
            tc.tile_pool(name="ps3", bufs=3, space=bass.MemorySpace.PSUM))
        pO = ctx.enter_context(
            tc.tile_pool(name="po1", bufs=1, space=bass.MemorySpace.PSUM))

        # ---- constants / weights in SBUF -------------------------------
        ident = pc.tile([P, P], BF16, tag="ident")
        make_identity(nc, ident[:])
        ones2 = pc.tile([P, 2, 1], FP8, tag="ones2")
        nc.vector.memset(ones2[:], 1.0)
        negone = pc.tile([P, 1], F32, tag="negone")
        nc.vector.memset(negone[:], -1.0)
        scratch = pc.tile([P, 512], BF16, tag="scratch")
        nc.gpsimd.memset(scratch[:], 0.0)
        sqjunk = pc.tile([P, C], BF16, tag="sqjunk")

        # prime the ACT function table with the one table that serves
        # Sign/Identity/Copy AND Gelu, so no mid-kernel table re-load.
        nc.scalar.activation(sqjunk[:, 0:1], negone[:], GELU_FN)

        # x first (LN1 is the critical path), per-token-tile chunks
        xt = px.tile([P, nt, C], F32, tag="x")
        x_r = x_d.rearrange("(t p) c -> t p c", p=P)
        for t in range(nt):
            nc.sync.dma_start(xt[:, t, :], x_r[t])

        # qkv weights per-k-chunk so the first matmuls can start early
        wqkvT = pwbig.tile([P, CT, OC], FP8, tag="wbig")
        wq_r = wqkvT_d.rearrange("(k p) o -> k p o", p=P)
        for ci in range(CT):
            nc.sync.dma_start(wqkvT[:, ci, :], wq_r[ci])

        b1s = pc.tile([P, HT], F32, tag="b1s")
        nc.sync.dma_start(b1s[:], b1s_d)
        wpT = pwp.tile([P, CT, C], FP8, tag="wp")
        nc.sync.dma_start(wpT[:], wpT_d.rearrange("(k p) o -> p k o", p=P))
        opt = {}
        for name, d in opt_d.items():
            opt[name] = pc.tile([P, C], F32, tag=name, name=f"t_{name}")
            nc.sync.dma_start(opt[name][:], d)

        # HAM warm-up: full-tile matmuls on zeros while LN1 runs
        # (PE is otherwise idle and starts the real work at 1.2 GHz).
        warm_n = [0]

        def warm(k=1):
            for _ in range(k):
                wp = pS.tile([P, 512], F32, tag="S",
                             name=f"warm{warm_n[0]}")
                warm_n[0] += 1
                nc.tensor.matmul(wp[:], lhsT=scratch[:, 0:P], rhs=scratch[:],
                                 start=True, stop=True)

        warm(8)

        # ---- stats tiles ------------------------------------------------
        musum = pstat.tile([P, nt], F32, tag="musum")
        nmu1 = pstat.tile([P, nt], F32, tag="nmu1")
        bn6 = pstat.tile([P, 2, 6], F32, tag="bn6")
        mv = pstat.tile([P, 2 * nt], F32, tag="mv")
        nmu2 = pstat.tile([P, nt], F32, tag="nmu2")
        r2 = pstat.tile([P, nt], F32, tag="r2")
        nmr2 = pstat.tile([P, nt], F32, tag="nmr2")
        rs_a = pstat.tile([P, nt], F32, tag="rs_a")
        rs_b = pstat.tile([P, nt], F32, tag="rs_b")
        if not cfg.ln1_fast:
            r1 = pstat.tile([P, nt], F32, tag="r1")
            mv1 = pstat.tile([P, 2 * nt], F32, tag="mv1")
            lntmp = ptok.tile([P, nt, C], F32, tag="lntmp")

        def rsqrt_cols(dst, var_col, t, newton=1):
            """dst[:, t:t+1] = 1/sqrt(var_col + eps), bit-trick + Newton.

            One Newton step gives ~0.17% rel error; even the raw bit-trick
            (~3.4%) only perturbs the normalized h2 scale, which reaches the
            output through the 1e-5 layer-scale branch -- far below tol."""
            a = rs_a[:, t:t + 1]
            b = rs_b[:, t:t + 1]
            nc.vector.tensor_scalar_add(a, var_col, 1e-5)          # v
            ai = a.bitcast(mybir.dt.int32)
            bi = b.bitcast(mybir.dt.int32)
            nc.vector.tensor_scalar(bi, ai, 1, None, op0=AL.arith_shift_right)
            nc.vector.tensor_scalar(bi, bi, -1, 0x5F3759DF, op0=AL.mult, op1=AL.add)
            if not newton:
                nc.vector.tensor_copy(dst, b)
                return
            # Newton: y1 = y0*(1.5 - 0.5*v*y0^2), y0 = b, result -> dst
            nc.vector.tensor_tensor(dst, b, b, op=AL.mult)
            nc.vector.tensor_tensor(dst, dst, a, op=AL.mult)
            nc.vector.tensor_scalar(dst, dst, -0.5, 1.5, op0=AL.mult, op1=AL.add)
            nc.vector.tensor_tensor(dst, dst, b, op=AL.mult)

        def hi_bf16(ps_ap):
            """View the bf16 high halves of an fp32 psum AP (exact for the
            small-integer matmul outputs binarized below; 16-bit reads run
            the DVE data path at 2x)."""
            return ps_ap.bitcast(BF16).rearrange("p (n two) -> p n two",
                                                 two=2)[:, :, 1]

        _sc = nc.enter_named_scope("ln1", False)
        # ---- LN1 -> s1 = sign(.) ; s1T transposes -----------------------
        s1 = ptok.tile([P, nt, C], BF16, tag="tok")
        s1T = pch.tile([P, CT, N], FP8, tag="ch")
        if not cfg.ln1_fast:
            g1r, b1r = opt["g1r"], opt["b1r"]

        def tr_tile(t, src, dstT, pfx):
            # transpose token tile t into one psum slot, single wide evac
            trs = pS.tile([P, CT, P], BF16, tag="S", name=f"{pfx}{t}")
            for ci in range(CT):
                nc.tensor.transpose(trs[:, ci, :], src[:, t, ci * P:(ci + 1) * P],
                                    ident[:])
            dst = dstT[:, :, t * P:(t + 1) * P]
            if t % 2:
                nc.vector.tensor_copy(dst, trs[:])
            else:
                nc.scalar.copy(dst, trs[:])

        # software-pipelined by one tile: PE transposes tile t-1 while the
        # ACT/DVE chain for tile t runs, so the serial LN chain never
        # stalls the PE.
        for t in range(nt):
            x_t = xt[:, t, :]
            if cfg.ln1_fast:
                nc.vector.tensor_reduce(musum[:, t:t + 1], x_t,
                                        axis=mybir.AxisListType.X, op=AL.add)
                nc.vector.tensor_scalar_mul(nmu1[:, t:t + 1], musum[:, t:t + 1],
                                            -1.0 / C)
                nc.scalar.activation(s1[:, t, :], x_t, AF.Sign,
                                     bias=nmu1[:, t:t + 1], scale=1.0)
            else:
                nc.vector.bn_stats(bn6[:, 0, :], x_t[:, :C // 2])
                nc.vector.bn_stats(bn6[:, 1, :], x_t[:, C // 2:])
                nc.vector.bn_aggr(mv1[:, 2 * t:2 * t + 2], bn6[:])
                rsqrt_cols(r1[:, t:t + 1], mv1[:, 2 * t + 1:2 * t + 2], t)
                nc.vector.tensor_scalar_mul(nmu1[:, t:t + 1],
                                            mv1[:, 2 * t:2 * t + 1], -1.0)
                u = lntmp[:, t, :]
                nc.vector.tensor_scalar(u, x_t, nmu1[:, t:t + 1], r1[:, t:t + 1],
                                        op0=AL.add, op1=AL.mult)
                nc.vector.tensor_tensor(u, u, g1r[:], op=AL.mult)
                nc.vector.tensor_tensor(u, u, b1r[:], op=AL.add)
                nc.scalar.activation(s1[:, t, :], u, AF.Sign, bias=0.0, scale=1.0)
            if t > 0:
                tr_tile(t - 1, s1, s1T, "tr1_")
            warm(1)
        tr_tile(nt - 1, s1, s1T, "tr1_")

        # ---- qkv: z^T for q,k sections (o-major), z for v (n-major) -----
        nc.leave_named_scope("ln1", _sc[0] if isinstance(_sc, tuple) else _sc, False)
        dbg_dump(nc, "s1T", s1T[:])

        # q kept full-tile (both heads of a pair stacked on partitions);
        # k stored zero-padded per head on the contraction (partition) dim:
        # kza[:, p] = [k_h0^T ; 0], kzb[:, p] = [0 ; k_h1^T].  S matmuls
        # then run full-K (128) against the full q tile -- the zero rows
        # kill the other head's contribution -- which keeps the HAM
        # activity monitor warm (sub-array tile_position matmuls do not
        # register as PE-busy and the whole phase gets clock-gated to
        # 1.2 GHz otherwise).
        qkT = pqk.tile([P, H // 2, N], FP8, tag="qk")
        kza = pqk.tile([P, H // 2, N], FP8, tag="kza")
        kzb = pqk.tile([P, H // 2, N], FP8, tag="kzb")
        nc.gpsimd.memset(kza[HD:P, :, :], 0.0)
        nc.gpsimd.memset(kzb[0:HD, :, :], 0.0)

        # DVE two-op +-1 binarize (Sign(2z+1) = (z > -0.5)*2 - 1) used to
        # offload part of the evacuation load from the (busier) ScalarE.
        zt8 = pc.tile([P, N], FP8, tag="zt8")

        def sign_evac_dve(dst, src):
            hi = hi_bf16(src)
            nc.vector.tensor_scalar(zt8[:, :src.free_size()], hi, -0.5, 2.0,
                                    op0=AL.is_gt, op1=AL.mult)
            nc.vector.tensor_scalar_add(dst, zt8[:, :src.free_size()], -1.0)

        for p_ in range(H // 2):
            for ot in (p_, 6 + p_):  # q tile p_, then k tile p_
                ps = pS.tile([P, N], F32, tag="S", name=f"zq{ot}")
                for (n0, nsz) in NCH:
                    for j in range(CT // 2):
                        nc.tensor.matmul(
                            ps[:, n0:n0 + nsz],
                            lhsT=wqkvT[:, 2 * j:2 * j + 2, ot * P:(ot + 1) * P],
                            rhs=s1T[:, 2 * j:2 * j + 2, n0:n0 + nsz],
                            start=(j == 0), stop=(j == CT // 2 - 1),
                            perf_mode=DR)
                if ot < 6:
                    if p_ % 2:
                        sign_evac_dve(qkT[:, ot, :], ps[:])
                    else:
                        nc.scalar.activation(qkT[:, ot, :], ps[:], AF.Sign,
                                             bias=1.0, scale=2.0)
                else:
                    nc.scalar.activation(kza[0:HD, p_, :], ps[0:HD, :],
                                         AF.Sign, bias=1.0, scale=2.0)
                    nc.scalar.activation(kzb[HD:P, p_, :], ps[HD:P, :],
                                         AF.Sign, bias=1.0, scale=2.0)

        # v, zero-padded per head on the stationary (free) dim so the O
        # matmuls are full-M: vz[:, :, 0, p, :] = [v_h0 | 0],
        # vz[:, :, 1, p, :] = [0 | v_h1]; the pair's two heads then
        # accumulate into ONE psum bank as [O_h0^T ; 0] + [0 ; O_h1^T].
        vz = pv.tile([P, nt, 2, H // 2, P], FP8, tag="vz")
        nc.gpsimd.memset(vz[:], 0.0)
        for t in range(nt):
            ps = pS.tile([P, C], F32, tag="S", name=f"zv{t}")
            for (o0, osz) in _nchunks(C):
                for j in range(CT // 2):
                    nc.tensor.matmul(
                        ps[:, o0:o0 + osz],
                        lhsT=s1T[:, 2 * j:2 * j + 2, t * P:(t + 1) * P],
                        rhs=wqkvT[:, 2 * j:2 * j + 2,
                                  2 * C + o0:2 * C + o0 + osz],
                        start=(j == 0), stop=(j == CT // 2 - 1), perf_mode=DR)
            # psum cols = 12 heads x 64; even heads -> vz[..,0,pair,0:64],
            # odd heads -> vz[..,1,pair,64:128]; one wide evac per parity
            ps_v = ps[:, 0:C].rearrange("p (h d) -> p h d", d=HD)
            if t % 3 == 1:
                zt8v = zt8[:, 0:C // 2].rearrange("p (h d) -> p h d", d=HD)
                hi_v = hi_bf16(ps[:, 0:C]).rearrange("p (h d) -> p h d", d=HD)
                for par in (0, 1):
                    dst = vz[:, t, par, :, par * HD:par * HD + HD]
                    nc.vector.tensor_scalar(zt8v, hi_v[:, par::2, :], -0.5, 2.0,
                                            op0=AL.is_gt, op1=AL.mult)
                    nc.vector.tensor_scalar_add(dst, zt8v, -1.0)
            else:
                nc.scalar.activation(vz[:, t, 0, :, 0:HD], ps_v[:, 0::2, :],
                                     AF.Sign, bias=1.0, scale=2.0)
                nc.scalar.activation(vz[:, t, 1, :, HD:P], ps_v[:, 1::2, :],
                                     AF.Sign, bias=1.0, scale=2.0)

        if dbg:
            dbg_dump(nc, "qkT", qkT[:])
            dbg_dump(nc, "kza", kza[:])
            dbg_dump(nc, "kzb", kzb[:])
            dbg_dump(nc, "vz", vz[:])

        # fc1 weights arrive during attention (free slot of the wbig pool)
        w1T = pwbig.tile([P, CT, HID], FP8, tag="wbig")
        nc.sync.dma_start(w1T[:], w1T_d.rearrange("(k p) o -> p k o", p=P))

        # ---- colsum of v per head (bias for +-1-encoded heads) ----------
        # cb_all[:, p] = sum_m v[m, c] + 1 for c-tile p (c = head*64+d),
        # memset to 1.0 for {0,2}-encoded head halves.  DoubleRow over
        # nt-chunk pairs: 4 matmuls per pair instead of 8.
        cb_all = pc.tile([P, H // 2], F32, tag="cball")
        for p_ in range(H // 2):
            h0in = 2 * p_ in ACT_HEADS
            h1in = 2 * p_ + 1 in ACT_HEADS
            if h0in or h1in:
                csp = pS.tile([P, 1], F32, tag="S", name=f"csp{p_}")
                srcs = ([0] if h0in else []) + ([1] if h1in else [])
                tot = (nt // 2) * len(srcs)
                nmm = 0
                for q in range(nt // 2):
                    for hh in srcs:
                        nc.tensor.matmul(csp[:], lhsT=vz[:, 2 * q:2 * q + 2, hh, p_, :],
                                         rhs=ones2[:], start=(nmm == 0),
                                         stop=(nmm == tot - 1), perf_mode=DR)
                        nmm += 1
                nc.scalar.activation(cb_all[:, p_:p_ + 1], csp[:],
                                     AF.Identity, bias=1.0, scale=1.0)
                if not h0in:
                    nc.vector.memset(cb_all[0:HD, p_:p_ + 1], 1.0)
                if not h1in:
                    nc.vector.memset(cb_all[HD:P, p_:p_ + 1], 1.0)
            else:
                nc.vector.memset(cb_all[:, p_:p_ + 1], 1.0)

        # ---- attention: software-pipelined S(p+1) before O(p) -----------
        soT = pch.tile([P, CT, N], FP8, tag="ch")
        n_pairs = H // 2
        st_tiles = {}

        def alloc_S(p_):
            st0 = pst.tile([P, nt, N], FP8, tag="st", name=f"st{2 * p_}")
            st1 = pst.tile([P, nt, N], FP8, tag="st", name=f"st{2 * p_ + 1}")
            st_tiles[p_] = (st0, st1)

        def emit_S_mt(p_, mt):
            # 2x2 tile_position packing: K is only 64 per head, so both
            # heads' S tiles for this mt run concurrently in the four
            # 64x64 quadrants of the PE array (2x S throughput).  The
            # interleaved full-array O matmuls keep the HAM activity
            # monitor fed through this sub-array stretch.
            st0, st1 = st_tiles[p_]
            m0 = mt * P
            ps0 = pS.tile([P, N], F32, tag="S", name=f"sps{2 * p_}_{mt}")
            ps1 = pS.tile([P, N], F32, tag="S", name=f"sps{2 * p_ + 1}_{mt}")
            for (n0, nsz) in NCH:
                nc.tensor.matmul(
                    ps0[0:HD, n0:n0 + nsz], lhsT=kza[0:HD, p_, m0:m0 + HD],
                    rhs=qkT[0:HD, p_, n0:n0 + nsz], start=True, stop=True,
                    tile_position=(0, 0))
                nc.tensor.matmul(
                    ps0[HD:P, n0:n0 + nsz], lhsT=kza[0:HD, p_, m0 + HD:m0 + P],
                    rhs=qkT[0:HD, p_, n0:n0 + nsz], start=True, stop=True,
                    tile_position=(0, 64))
                nc.tensor.matmul(
                    ps1[0:HD, n0:n0 + nsz], lhsT=kzb[HD:P, p_, m0:m0 + HD],
                    rhs=qkT[HD:P, p_, n0:n0 + nsz], start=True, stop=True,
                    tile_position=(64, 0))
                nc.tensor.matmul(
                    ps1[HD:P, n0:n0 + nsz], lhsT=kzb[HD:P, p_, m0 + HD:m0 + P],
                    rhs=qkT[HD:P, p_, n0:n0 + nsz], start=True, stop=True,
                    tile_position=(64, 64))
            for hh, ps, st in ((0, ps0, st0), (1, ps1, st1)):
                head = 2 * p_ + hh
                if head in ACT_HEADS:
                    # +-1 encoding: Sign(S-1); S even => never 0
                    nc.scalar.activation(st[:, mt, :], ps[:], AF.Sign,
                                         bias=negone[:, 0:1], scale=1.0)
                else:
                    # {0,2} encoding: (S>0)*2; S is small-integer so the
                    # bf16 high halves are exact and read at 2x
                    nc.vector.tensor_scalar(st[:, mt, :], hi_bf16(ps[:]), 0.0, 2.0,
                                            op0=AL.is_gt, op1=AL.mult)

        ot_tiles = {}

        def emit_O_j(p_, j):
            # one psum bank per n-chunk; both heads accumulate into it
            # ([O_h0^T ; 0] + [0 ; O_h1^T]) with full-M DoubleRow matmuls.
            st0, st1 = st_tiles[p_]
            if j == 0:
                ot_tiles[p_] = pO.tile([P, N], F32, tag="oacc",
                                       name=f"ot{p_}")
            ots = ot_tiles[p_]
            nj = nt // 2
            for hh, st in ((0, st0), (1, st1)):
                for (n0, nsz) in NCH:
                    nc.tensor.matmul(
                        ots[:, n0:n0 + nsz],
                        lhsT=vz[:, 2 * j:2 * j + 2, hh, p_, :],
                        rhs=st[:, 2 * j:2 * j + 2, n0:n0 + nsz],
                        start=(j == 0 and hh == 0),
                        stop=(j == nj - 1 and hh == 1), perf_mode=DR)

        def emit_O_tail(p_):
            st_tiles.pop(p_)
            ots = ot_tiles.pop(p_)
            nc.scalar.activation(soT[:, p_, :], ots[:], AF.Sign,
                                 bias=cb_all[:, p_:p_ + 1], scale=1.0)

        def dbg_dump_st(p_):
            if not dbg:
                return
            st0, st1 = st_tiles[p_]
            dbg_dump(nc, f"st{2 * p_}", st0[:])
            dbg_dump(nc, f"st{2 * p_ + 1}", st1[:])

        # software pipeline at mt granularity: while pair p_'s S tiles are
        # produced (gated by the binarize evacs), the previous pair's O
        # matmuls are interleaved in the PE stream so the engine never
        # stalls behind a pending evacuation.
        with nc.named_scope("attn"):
            alloc_S(0)
            for mt in range(nt):
                emit_S_mt(0, mt)
            dbg_dump_st(0)
            # fc2 weights arrive during attention (wqkvT's slot is free now)
            w2T = pwbig.tile([P, HT, C], FP8, tag="wbig")
            nc.sync.dma_start(w2T[:], w2T_d.rearrange("(k p) o -> p k o", p=P))
            for p_ in range(1, n_pairs):
                alloc_S(p_)
                for mt in range(nt):
                    emit_S_mt(p_, mt)
                    if mt % 2 == 1:
                        emit_O_j(p_ - 1, mt // 2)
                dbg_dump_st(p_)
                emit_O_tail(p_ - 1)
            for j in range(nt // 2):
                emit_O_j(n_pairs - 1, j)
            emit_O_tail(n_pairs - 1)
        dbg_dump(nc, "cball", cb_all[:])
        dbg_dump(nc, "soT", soT[:])

        # ---- proj + residual + LN2 (per token tile, interleaved) --------
        h2 = ptok.tile([P, nt, C], BF16, tag="tok")
        h2T = pch.tile([P, CT, N], FP8, tag="ch")
        if not cfg.ln2_fast:
            g2r, b2r = opt["g2r"], opt["b2r"]
            h2f = ptok.tile([P, nt, C], F32, tag="h2f")

        def ln2_tail(t):
            # LN2 stats + h2 + transposes for tile t (runs one tile behind
            # the proj matmuls so PE never waits on this serial chain).
            x_t = xt[:, t, :]
            nc.vector.bn_stats(bn6[:, 0, :], x_t[:, :C // 2])
            nc.vector.bn_stats(bn6[:, 1, :], x_t[:, C // 2:])
            nc.vector.bn_aggr(mv[:, 2 * t:2 * t + 2], bn6[:])
            rsqrt_cols(r2[:, t:t + 1], mv[:, 2 * t + 1:2 * t + 2], t)
            nc.vector.tensor_scalar_mul(nmu2[:, t:t + 1], mv[:, 2 * t:2 * t + 1],
                                        -1.0)
            nc.vector.tensor_tensor(nmr2[:, t:t + 1], nmu2[:, t:t + 1],
                                    r2[:, t:t + 1], op=AL.mult)
            if cfg.ln2_fast:
                # h2 = (x1 - mu)*r on ScalarE: func(x*scale + bias)
                nc.scalar.activation(h2[:, t, :], x_t, AF.Identity,
                                     bias=nmr2[:, t:t + 1],
                                     scale=r2[:, t:t + 1])
            else:
                u = h2f[:, t, :]
                nc.vector.tensor_scalar(u, x_t, nmu2[:, t:t + 1], r2[:, t:t + 1],
                                        op0=AL.add, op1=AL.mult)
                nc.vector.tensor_tensor(u, u, g2r[:], op=AL.mult)
                nc.vector.tensor_tensor(h2[:, t, :], u, b2r[:], op=AL.add)


        for t in range(nt):
            ps = pS.tile([P, C], F32, tag="S", name=f"prj{t}")
            for (o0, osz) in _nchunks(C):
                for j in range(CT // 2):
                    nc.tensor.matmul(
                        ps[:, o0:o0 + osz],
                        lhsT=soT[:, 2 * j:2 * j + 2, t * P:(t + 1) * P],
                        rhs=wpT[:, 2 * j:2 * j + 2, o0:o0 + osz],
                        start=(j == 0), stop=(j == CT // 2 - 1), perf_mode=DR)
            x_t = xt[:, t, :]
            # x1 = x + psum * ls1/64  (alpha_p*64 folded into wpT), or the
            # general per-channel path when ls1 is not uniform.
            if uni1:
                nc.scalar.activation(ps[:], ps[:], AF.Identity,
                                     bias=0.0, scale=cfg.ls1s)
            else:
                nc.vector.tensor_tensor(ps[:], ps[:], opt["cp1r"][:], op=AL.mult)
            nc.vector.tensor_tensor(x_t, x_t, ps[:], op=AL.add)
            if cfg.has_cp2:
                nc.vector.tensor_tensor(x_t, x_t, opt["cp2r"][:], op=AL.add)
            if t > 0:
                ln2_tail(t - 1)
            if t > 1:
                tr_tile(t - 2, h2, h2T, "tr2_")
        ln2_tail(nt - 1)
        tr_tile(nt - 2, h2, h2T, "tr2_")
        tr_tile(nt - 1, h2, h2T, "tr2_")

        dbg_dump(nc, "x1", xt[:])
        dbg_dump(nc, "h2", h2[:])

        # ---- fc1 -> gelu -> mgT (h-major) -------------------------------
        # w1T carries a1*64; gelu arg = psum/64 + b1.
        mgT = [pst.tile([P, 8, N], FP8, tag="st", name=f"mgT{j}")
               for j in range((HT + 7) // 8)]
        for ht in range(HT):
            ps = pS.tile([P, N], F32, tag="S", name=f"f1_{ht}")
            for (n0, nsz) in NCH:
                for j in range(CT // 2):
                    nc.tensor.matmul(
                        ps[:, n0:n0 + nsz],
                        lhsT=w1T[:, 2 * j:2 * j + 2, ht * P:(ht + 1) * P],
                        rhs=h2T[:, 2 * j:2 * j + 2, n0:n0 + nsz],
                        start=(j == 0), stop=(j == CT // 2 - 1), perf_mode=DR)
            nc.scalar.activation(mgT[ht // 8][:, ht % 8, :], ps[:], GELU_FN,
                                 bias=b1s[:, ht:ht + 1],
                                 scale=1.0 / 64.0)

        if dbg:
            for j, mg in enumerate(mgT):
                dbg_dump(nc, f"mgT{j}", mg[:])

        # ---- fc2 + residual -> out --------------------------------------
        for t in range(nt):
            ps = pS.tile([P, C], F32, tag="S", name=f"f2_{t}")
            for (o0, osz) in _nchunks(C):
                for j in range(HT // 2):
                    mg = mgT[j // 4]
                    k0 = (j % 4) * 2
                    nc.tensor.matmul(
                        ps[:, o0:o0 + osz],
                        lhsT=mg[:, k0:k0 + 2, t * P:(t + 1) * P],
                        rhs=w2T[:, 2 * j:2 * j + 2, o0:o0 + osz],
                        start=(j == 0), stop=(j == HT // 2 - 1), perf_mode=DR)
            x_t = xt[:, t, :]
            # out = x1 + psum * ls2/64 (alpha2*64 folded into w2T)
            if uni2:
                nc.scalar.activation(ps[:], ps[:], AF.Identity,
                                     bias=0.0, scale=cfg.ls2s)
            else:
                nc.vector.tensor_tensor(ps[:], ps[:], opt["c1r"][:], op=AL.mult)
            nc.vector.tensor_tensor(x_t, x_t, ps[:], op=AL.add)
            if cfg.has_c2:
                nc.vector.tensor_tensor(x_t, x_t, opt["c2r"][:], op=AL.add)
            nc.sync.dma_start(
                out_d.rearrange("(t p) c -> t p c", p=P)[t], x_t)

    nc.compile()
    input_names = ["x", "wqkvT", "wpT", "w1T", "w2T", "b1s"] + list(opt_d.keys())
    if dbg:
        return nc, input_names, dbg_t
    return nc, input_names


# -------------------------------------------------------------------------
# host-side prep + execution
# -------------------------------------------------------------------------

def _sgn(a):
    return np.where(a >= 0, np.float32(1.0), np.float32(-1.0))


def prep_host_inputs(inputs, cfg: Cfg):
    """Returns dict of per-core-common host arrays keyed by dram names."""
    f8 = ml_dtypes.float8_e4m3
    w_qkv = np.asarray(inputs["w_qkv"], np.float32)
    w_proj = np.asarray(inputs["w_proj"], np.float32)
    w_fc1 = np.asarray(inputs["w_fc1"], np.float32)
    w_fc2 = np.asarray(inputs["w_fc2"], np.float32)
    ls1 = np.asarray(inputs["ls1_g"], np.float32)
    ls2 = np.asarray(inputs["ls2_g"], np.float32)
    b_proj = np.asarray(inputs["b_proj"], np.float32)
    b_fc1 = np.asarray(inputs["b_fc1"], np.float32)
    b_fc2 = np.asarray(inputs["b_fc2"], np.float32)

    ap = np.abs(w_proj).mean(axis=1)    # [C] alpha_proj
    a1 = np.abs(w_fc1).mean(axis=1)     # [HID]
    a2 = np.abs(w_fc2).mean(axis=1)     # [C]

    uni1 = cfg.ls1s != 0.0
    uni2 = cfg.ls2s != 0.0
    # alpha*64 folded into the fp8 sign weights (x64 keeps the values in
    # fp8e4m3 normal range; the rel. quantization error ~6% enters the
    # output only through the 1e-5-scaled residual branches).
    wp_scale = (ap * 64.0)[None, :] if uni1 else np.float32(1.0)
    w1_scale = (a1 * 64.0)[:, None]
    w2_scale = (a2 * 64.0)[:, None] if uni2 else np.float32(1.0)

    d = {
        "wqkvT": np.ascontiguousarray(_sgn(w_qkv).T).astype(f8),
        "wpT": np.ascontiguousarray(_sgn(w_proj).T * wp_scale).astype(f8),
        "w1T": np.ascontiguousarray((_sgn(w_fc1) * w1_scale).T).astype(f8),
        "w2T": np.ascontiguousarray((_sgn(w_fc2) * w2_scale).T).astype(f8),
        "b1s": np.ascontiguousarray(b_fc1.reshape(HT, P).T),
    }
    if not uni1:
        d["cp1r"] = np.ascontiguousarray(
            np.broadcast_to(ls1 * ap, (P, C)).copy())
        # w1 always folds a1; gelu scale 1/64 stays correct either way
    if not uni2:
        d["c1r"] = np.ascontiguousarray(
            np.broadcast_to(ls2 * a2, (P, C)).copy())
    if cfg.has_cp2:
        d["cp2r"] = np.ascontiguousarray(np.broadcast_to(ls1 * b_proj, (P, C)).copy())
    if cfg.has_c2:
        d["c2r"] = np.ascontiguousarray(np.broadcast_to(ls2 * b_fc2, (P, C)).copy())
    if not cfg.ln1_fast:
        d["g1r"] = np.ascontiguousarray(
            np.broadcast_to(np.asarray(inputs["ln1_g"], np.float32), (P, C)).copy())
        d["b1r"] = np.ascontiguousarray(
            np.broadcast_to(np.asarray(inputs["ln1_b"], np.float32), (P, C)).copy())
    if not cfg.ln2_fast:
        d["g2r"] = np.ascontiguousarray(
            np.broadcast_to(np.asarray(inputs["ln2_g"], np.float32), (P, C)).copy())
        d["b2r"] = np.ascontiguousarray(
            np.broadcast_to(np.asarray(inputs["ln2_b"], np.float32), (P, C)).copy())
    return d


def make_cfg(inputs, nt=8):
    ln1_g = np.asarray(inputs["ln1_g"], np.float32)
    ln1_b = np.asarray(inputs["ln1_b"], np.float32)
    ln2_g = np.asarray(inputs["ln2_g"], np.float32)
    ln2_b = np.asarray(inputs["ln2_b"], np.float32)
    ls1 = np.asarray(inputs["ls1_g"], np.float32)
    ls2 = np.asarray(inputs["ls2_g"], np.float32)
    b_proj = np.asarray(inputs["b_proj"], np.float32)
    b_fc2 = np.asarray(inputs["b_fc2"], np.float32)
    uni1 = bool(np.all(ls1 == ls1[0]))
    uni2 = bool(np.all(ls2 == ls2[0]))
    return Cfg(
        nt=nt,
        ln1_fast=bool(np.all(ln1_b == 0) and np.all(ln1_g > 0)),
        ln2_fast=bool(np.all(ln2_g == 1) and np.all(ln2_b == 0)),
        has_cp2=bool(np.any(ls1 * b_proj != 0)),
        has_c2=bool(np.any(ls2 * b_fc2 != 0)),
        ls1s=float(ls1[0]) / 64.0 if uni1 else 0.0,
        ls2s=float(ls2[0]) / 64.0 if uni2 else 0.0,
    )


_PROG_CACHE = {}


def get_program(cfg: Cfg):
    key = cfg
    if key not in _PROG_CACHE:
        _PROG_CACHE[key] = build_program(cfg)
    return _PROG_CACHE[key]


def kernel(**inputs):
    from concourse.bass_utils import run_bass_kernel_spmd

    x = np.asarray(inputs["x"], np.float32)
    assert x.shape == (B, 1024, C), x.shape
    cfg = make_cfg(inputs, nt=1024 // P)
    nc, _names = get_program(cfg)
    common = prep_host_inputs(inputs, cfg)

    in_maps = []
    for b in range(B):
        m = dict(common)
        m["x"] = np.ascontiguousarray(x[b])
        in_maps.append(m)

    res = run_bass_kernel_spmd(nc, in_maps, core_ids=list(range(N_CORES)))
    out = np.stack([res.results[b]["out"] for b in range(B)], axis=0)
    return out.astype(np.float32)


# revision 11
# speedup vs baseline: 1.1018x; 1.1018x over previous
"""Trainium2 Bass kernel for a binarized transformer block (BiT-style).

Block (per batch element, forward only):
    h   = LN1(x);  s1 = sign(h)
    z   = s1 @ sign(w_qkv)^T          (alpha>0 dropped: only signs consumed)
    q,k,v = sign(z) split into heads  (+-1)
    S   = q @ k^T  (integer);  T = (S>0)   <- forward value of softmax-STE
    O   = T @ v    (integer);  so = sign(O)
    x1  = x + ls1*(so @ (a_p*sign(w_proj))^T + b_proj)
    h2  = LN2(x1)
    m   = gelu(h2 @ sign(w_fc1)^T * a1 + b1)
    out = x1 + ls2*(m @ (a2*sign(w_fc2))^T + b_fc2)

All binary matmuls are exact: +-1/{0,2} operands in fp8, fp32 PSUM
accumulation of integers.  Thresholds are Sign(2z+1) on odd integers, so
never evaluated at 0.  The proj/fc1/fc2 weights carry their per-channel
alpha scales (x64, fp8-rounded) folded in; the residual adds then use a
single scalar ls/64 scale when the layer-scale vector is uniform.
Sharding: batch 8 -> one element per NeuronCore, no collectives.
"""

import sys
import os

sys.path.insert(0, "/opt/trn_rl_repo")

import numpy as np
import ml_dtypes
from contextlib import ExitStack
from dataclasses import dataclass

from concourse import bass, bacc, mybir, tile
from concourse.masks import make_identity

P = 128
C = 768
CT = C // P          # 6 channel chunks
H = 12
HD = 64
HID = 3072
HT = HID // P        # 24 hidden chunks
OC = 3 * C           # 2304
B = 8
N_CORES = 8

F32 = mybir.dt.float32
BF16 = mybir.dt.bfloat16
FP8 = mybir.dt.float8e4
AF = mybir.ActivationFunctionType
AL = mybir.AluOpType

# heads whose S-binarize runs on ScalarE (+-1 encoding, colsum-corrected);
# the rest run on VectorE ({0,2} encoding, direct).
ACT_HEADS = frozenset(range(0, 12, 2))
DR = mybir.MatmulPerfMode.DoubleRow

# dev hook: CoreSim has no Gelu; dev_sim swaps this for Tanh on both sides.
GELU_FN = AF.Gelu


@dataclass(frozen=True)
class Cfg:
    nt: int = 8            # token tiles of 128 per core
    ln1_fast: bool = True  # ln1_b == 0 and ln1_g > 0 elementwise
    ln2_fast: bool = True  # ln2_g == 1 and ln2_b == 0
    has_cp2: bool = False  # ls1*b_proj != 0
    has_c2: bool = False   # ls2*b_fc2 != 0
    # uniform layer scales -> alpha-folded weights + scalar residual scale
    ls1s: float = 0.0      # ls1/64 when uniform, else 0 (tensor path)
    ls2s: float = 0.0


def _nchunks(n, step=512):
    out = []
    i = 0
    while i < n:
        out.append((i, min(step, n - i)))
        i += step
    return out


def build_program(cfg: Cfg, dbg=False):
    """Builds the per-core Bass program. Returns (nc, input_names)."""
    nt = cfg.nt
    N = nt * P
    NCH = _nchunks(N)
    uni1 = cfg.ls1s != 0.0
    uni2 = cfg.ls2s != 0.0

    dbg_t = {}

    def dbg_dump(nc, name, ap):
        if not dbg:
            return
        d = nc.dram_tensor(f"dbg_{name}", list(ap.shape), ap.dtype,
                           kind="ExternalOutput").ap()
        dbg_t[name] = d
        nc.sync.dma_start(d, ap)

    nc = bacc.Bacc("TRN2", target_bir_lowering=False, debug=False,
                   enable_asserts=False, num_devices=N_CORES)

    # ---- DRAM I/O -------------------------------------------------------
    x_d = nc.dram_tensor("x", [N, C], F32, kind="ExternalInput").ap()
    wqkvT_d = nc.dram_tensor("wqkvT", [C, OC], FP8, kind="ExternalInput").ap()
    wpT_d = nc.dram_tensor("wpT", [C, C], FP8, kind="ExternalInput").ap()
    w1T_d = nc.dram_tensor("w1T", [C, HID], FP8, kind="ExternalInput").ap()
    w2T_d = nc.dram_tensor("w2T", [HID, C], FP8, kind="ExternalInput").ap()
    b1s_d = nc.dram_tensor("b1s", [P, HT], F32, kind="ExternalInput").ap()
    opt_d = {}
    if not uni1:
        opt_d["cp1r"] = nc.dram_tensor("cp1r", [P, C], F32, kind="ExternalInput").ap()
    if not uni2:
        opt_d["c1r"] = nc.dram_tensor("c1r", [P, C], F32, kind="ExternalInput").ap()
    if cfg.has_cp2:
        opt_d["cp2r"] = nc.dram_tensor("cp2r", [P, C], F32, kind="ExternalInput").ap()
    if cfg.has_c2:
        opt_d["c2r"] = nc.dram_tensor("c2r", [P, C], F32, kind="ExternalInput").ap()
    if not cfg.ln1_fast:
        opt_d["g1r"] = nc.dram_tensor("g1r", [P, C], F32, kind="ExternalInput").ap()
        opt_d["b1r"] = nc.dram_tensor("b1r", [P, C], F32, kind="ExternalInput").ap()
    if not cfg.ln2_fast:
        opt_d["g2r"] = nc.dram_tensor("g2r", [P, C], F32, kind="ExternalInput").ap()
        opt_d["b2r"] = nc.dram_tensor("b2r", [P, C], F32, kind="ExternalInput").ap()
    out_d = nc.dram_tensor("out", [N, C], F32, kind="ExternalOutput").ap()

    with tile.TileContext(nc) as tc, ExitStack() as ctx:
        pc = ctx.enter_context(tc.tile_pool(name="const", bufs=1))
        px = ctx.enter_context(tc.tile_pool(name="xp", bufs=1))
        pwbig = ctx.enter_context(tc.tile_pool(name="wbig", bufs=2))
        pwp = ctx.enter_context(tc.tile_pool(name="wp", bufs=1))
        ptok = ctx.enter_context(tc.tile_pool(name="tok", bufs=1))
        pch = ctx.enter_context(tc.tile_pool(name="ch", bufs=2))
        pqk = ctx.enter_context(tc.tile_pool(name="qk", bufs=1))
        pv = ctx.enter_context(tc.tile_pool(name="vp", bufs=1))
        pst = ctx.enter_context(tc.tile_pool(name="st", bufs=4))
        pstat = ctx.enter_context(tc.tile_pool(name="stat", bufs=1))

        # PSUM: 3 rotating double-bank (4KB) slots shared by every
        # accumulator / S tile / transpose batch / warm-up (tag "S"), plus
        # one dedicated slot for the attention O accumulator.  3 slots let
        # the PE run a full step ahead of the two evacuation engines.
        pS = ctx.enter_context(
            tc.tile_pool(name="ps3", bufs=3, space=bass.MemorySpace.PSUM))
        pO = ctx.enter_context(
            tc.tile_pool(name="po1", bufs=1, space=bass.MemorySpace.PSUM))

        # ---- constants / weights in SBUF -------------------------------
        ident = pc.tile([P, P], BF16, tag="ident")
        make_identity(nc, ident[:])
        ones2 = pc.tile([P, 2, 1], FP8, tag="ones2")
        nc.vector.memset(ones2[:], 1.0)
        negone = pc.tile([P, 1], F32, tag="negone")
        nc.vector.memset(negone[:], -1.0)
        scratch = pc.tile([P, 512], BF16, tag="scratch")
        nc.gpsimd.memset(scratch[:], 0.0)
        sqjunk = pc.tile([P, C], BF16, tag="sqjunk")

        # prime the ACT function table with the one table that serves
        # Sign/Identity/Copy AND Gelu, so no mid-kernel table re-load.
        nc.scalar.activation(sqjunk[:, 0:1], negone[:], GELU_FN)

        # x first (LN1 is the critical path), per-token-tile chunks
        xt = px.tile([P, nt, C], F32, tag="x")
        x_r = x_d.rearrange("(t p) c -> t p c", p=P)
        for t in range(nt):
            nc.sync.dma_start(xt[:, t, :], x_r[t])

        # qkv weights per-k-chunk so the first matmuls can start early
        wqkvT = pwbig.tile([P, CT, OC], FP8, tag="wbig")
        wq_r = wqkvT_d.rearrange("(k p) o -> k p o", p=P)
        for ci in range(CT):
            nc.sync.dma_start(wqkvT[:, ci, :], wq_r[ci])

        b1s = pc.tile([P, HT], F32, tag="b1s")
        nc.sync.dma_start(b1s[:], b1s_d)
        wpT = pwp.tile([P, CT, C], FP8, tag="wp")
        nc.sync.dma_start(wpT[:], wpT_d.rearrange("(k p) o -> p k o", p=P))
        opt = {}
        for name, d in opt_d.items():
            opt[name] = pc.tile([P, C], F32, tag=name, name=f"t_{name}")
            nc.sync.dma_start(opt[name][:], d)

        # HAM warm-up: full-tile matmuls on zeros while LN1 runs
        # (PE is otherwise idle and starts the real work at 1.2 GHz).
        warm_n = [0]

        def warm(k=1):
            for _ in range(k):
                wp = pS.tile([P, 512], F32, tag="S",
                             name=f"warm{warm_n[0]}")
                warm_n[0] += 1
                nc.tensor.matmul(wp[:], lhsT=scratch[:, 0:P], rhs=scratch[:],
                                 start=True, stop=True)

        warm(8)

        # ---- stats tiles ------------------------------------------------
        musum = pstat.tile([P, nt], F32, tag="musum")
        nmu1 = pstat.tile([P, nt], F32, tag="nmu1")
        bn6 = pstat.tile([P, 2, 6], F32, tag="bn6")
        mv = pstat.tile([P, 2 * nt], F32, tag="mv")
        nmu2 = pstat.tile([P, nt], F32, tag="nmu2")
        r2 = pstat.tile([P, nt], F32, tag="r2")
        nmr2 = pstat.tile([P, nt], F32, tag="nmr2")
        rs_a = pstat.tile([P, nt], F32, tag="rs_a")
        rs_b = pstat.tile([P, nt], F32, tag="rs_b")
        if not cfg.ln1_fast:
            r1 = pstat.tile([P, nt], F32, tag="r1")
            mv1 = pstat.tile([P, 2 * nt], F32, tag="mv1")
            lntmp = ptok.tile([P, nt, C], F32, tag="lntmp")

        def rsqrt_cols(dst, var_col, t, newton=1):
            """dst[:, t:t+1] = 1/sqrt(var_col + eps), bit-trick + Newton.

            One Newton step gives ~0.17% rel error; even the raw bit-trick
            (~3.4%) only perturbs the normalized h2 scale, which reaches the
            output through the 1e-5 layer-scale branch -- far below tol."""
            a = rs_a[:, t:t + 1]
            b = rs_b[:, t:t + 1]
            nc.vector.tensor_scalar_add(a, var_col, 1e-5)          # v
            ai = a.bitcast(mybir.dt.int32)
            bi = b.bitcast(mybir.dt.int32)
            nc.vector.tensor_scalar(bi, ai, 1, None, op0=AL.arith_shift_right)
            nc.vector.tensor_scalar(bi, bi, -1, 0x5F3759DF, op0=AL.mult, op1=AL.add)
            if not newton:
                nc.vector.tensor_copy(dst, b)
                return
            # Newton: y1 = y0*(1.5 - 0.5*v*y0^2), y0 = b, result -> dst
            nc.vector.tensor_tensor(dst, b, b, op=AL.mult)
            nc.vector.tensor_tensor(dst, dst, a, op=AL.mult)
            nc.vector.tensor_scalar(dst, dst, -0.5, 1.5, op0=AL.mult, op1=AL.add)
            nc.vector.tensor_tensor(dst, dst, b, op=AL.mult)

        def hi_bf16(ps_ap):
            """View the bf16 high halves of an fp32 psum AP (exact for the
            small-integer matmul outputs binarized below; 16-bit reads run
            the DVE data path at 2x)."""
            return ps_ap.bitcast(BF16).rearrange("p (n two) -> p n two",
                                                 two=2)[:, :, 1]

        _sc = nc.enter_named_scope("ln1", False)
        # ---- LN1 -> s1 = sign(.) ; s1T transposes -----------------------
        s1 = ptok.tile([P, nt, C], BF16, tag="tok")
        s1T = pch.tile([P, CT, N], FP8, tag="ch")
        if not cfg.ln1_fast:
            g1r, b1r = opt["g1r"], opt["b1r"]

        def tr_tile(t, src, dstT, pfx):
            # transpose token tile t into one psum slot, single wide evac
            trs = pS.tile([P, CT, P], BF16, tag="S", name=f"{pfx}{t}")
            for ci in range(CT):
                nc.tensor.transpose(trs[:, ci, :], src[:, t, ci * P:(ci + 1) * P],
                                    ident[:])
            dst = dstT[:, :, t * P:(t + 1) * P]
            if t % 2:
                nc.vector.tensor_copy(dst, trs[:])
            else:
                nc.scalar.copy(dst, trs[:])

        # software-pipelined by one tile: PE transposes tile t-1 while the
        # ACT/DVE chain for tile t runs, so the serial LN chain never
        # stalls the PE.
        for t in range(nt):
            x_t = xt[:, t, :]
            if cfg.ln1_fast:
                nc.vector.tensor_reduce(musum[:, t:t + 1], hi_bf16(x_t),
                                        axis=mybir.AxisListType.X, op=AL.add)
                nc.vector.tensor_scalar_mul(nmu1[:, t:t + 1], musum[:, t:t + 1],
                                            -1.0 / C)
                nc.scalar.activation(s1[:, t, :], hi_bf16(x_t), AF.Sign,
                                     bias=nmu1[:, t:t + 1], scale=1.0)
            else:
                nc.vector.bn_stats(bn6[:, 0, :], x_t[:, :C // 2])
                nc.vector.bn_stats(bn6[:, 1, :], x_t[:, C // 2:])
                nc.vector.bn_aggr(mv1[:, 2 * t:2 * t + 2], bn6[:])
                rsqrt_cols(r1[:, t:t + 1], mv1[:, 2 * t + 1:2 * t + 2], t)
                nc.vector.tensor_scalar_mul(nmu1[:, t:t + 1],
                                            mv1[:, 2 * t:2 * t + 1], -1.0)
                u = lntmp[:, t, :]
                nc.vector.tensor_scalar(u, x_t, nmu1[:, t:t + 1], r1[:, t:t + 1],
                                        op0=AL.add, op1=AL.mult)
                nc.vector.tensor_tensor(u, u, g1r[:], op=AL.mult)
                nc.vector.tensor_tensor(u, u, b1r[:], op=AL.add)
                nc.scalar.activation(s1[:, t, :], u, AF.Sign, bias=0.0, scale=1.0)
            if t > 0:
                tr_tile(t - 1, s1, s1T, "tr1_")
            warm(1)
        tr_tile(nt - 1, s1, s1T, "tr1_")

        # ---- qkv: z^T for q,k sections (o-major), z for v (n-major) -----
        nc.leave_named_scope("ln1", _sc[0] if isinstance(_sc, tuple) else _sc, False)
        dbg_dump(nc, "s1T", s1T[:])

        # q kept full-tile (both heads of a pair stacked on partitions);
        # k stored zero-padded per head on the contraction (partition) dim:
        # kza[:, p] = [k_h0^T ; 0], kzb[:, p] = [0 ; k_h1^T].  S matmuls
        # then run full-K (128) against the full q tile -- the zero rows
        # kill the other head's contribution -- which keeps the HAM
        # activity monitor warm (sub-array tile_position matmuls do not
        # register as PE-busy and the whole phase gets clock-gated to
        # 1.2 GHz otherwise).
        qkT = pqk.tile([P, H // 2, N], FP8, tag="qk")
        kza = pqk.tile([P, H // 2, N], FP8, tag="kza")
        kzb = pqk.tile([P, H // 2, N], FP8, tag="kzb")
        nc.gpsimd.memset(kza[HD:P, :, :], 0.0)
        nc.gpsimd.memset(kzb[0:HD, :, :], 0.0)

        # DVE two-op +-1 binarize (Sign(2z+1) = (z > -0.5)*2 - 1) used to
        # offload part of the evacuation load from the (busier) ScalarE.
        zt8 = pc.tile([P, N], FP8, tag="zt8")

        def sign_evac_dve(dst, src):
            hi = hi_bf16(src)
            nc.vector.tensor_scalar(zt8[:, :src.free_size()], hi, -0.5, 2.0,
                                    op0=AL.is_gt, op1=AL.mult)
            nc.vector.tensor_scalar_add(dst, zt8[:, :src.free_size()], -1.0)

        for p_ in range(H // 2):
            for ot in (p_, 6 + p_):  # q tile p_, then k tile p_
                ps = pS.tile([P, N], F32, tag="S", name=f"zq{ot}")
                for (n0, nsz) in NCH:
                    for j in range(CT // 2):
                        nc.tensor.matmul(
                            ps[:, n0:n0 + nsz],
                            lhsT=wqkvT[:, 2 * j:2 * j + 2, ot * P:(ot + 1) * P],
                            rhs=s1T[:, 2 * j:2 * j + 2, n0:n0 + nsz],
                            start=(j == 0), stop=(j == CT // 2 - 1),
                            perf_mode=DR)
                if ot < 6:
                    if p_ % 2:
                        sign_evac_dve(qkT[:, ot, :], ps[:])
                    else:
                        nc.scalar.activation(qkT[:, ot, :], hi_bf16(ps[:]),
                                             AF.Sign, bias=1.0, scale=2.0)
                else:
                    nc.scalar.activation(kza[0:HD, p_, :], hi_bf16(ps[0:HD, :]),
                                         AF.Sign, bias=1.0, scale=2.0)
                    nc.scalar.activation(kzb[HD:P, p_, :], hi_bf16(ps[HD:P, :]),
                                         AF.Sign, bias=1.0, scale=2.0)

        # v, zero-padded per head on the stationary (free) dim so the O
        # matmuls are full-M: vz[:, :, 0, p, :] = [v_h0 | 0],
        # vz[:, :, 1, p, :] = [0 | v_h1]; the pair's two heads then
        # accumulate into ONE psum bank as [O_h0^T ; 0] + [0 ; O_h1^T].
        vz = pv.tile([P, nt, 2, H // 2, P], FP8, tag="vz")
        nc.gpsimd.memset(vz[:], 0.0)
        for t in range(nt):
            ps = pS.tile([P, C], F32, tag="S", name=f"zv{t}")
            for (o0, osz) in _nchunks(C):
                for j in range(CT // 2):
                    nc.tensor.matmul(
                        ps[:, o0:o0 + osz],
                        lhsT=s1T[:, 2 * j:2 * j + 2, t * P:(t + 1) * P],
                        rhs=wqkvT[:, 2 * j:2 * j + 2,
                                  2 * C + o0:2 * C + o0 + osz],
                        start=(j == 0), stop=(j == CT // 2 - 1), perf_mode=DR)
            # psum cols = 12 heads x 64; even heads -> vz[..,0,pair,0:64],
            # odd heads -> vz[..,1,pair,64:128]; one wide evac per parity
            ps_v = ps[:, 0:C].rearrange("p (h d) -> p h d", d=HD)
            if t % 3 == 1:
                zt8v = zt8[:, 0:C // 2].rearrange("p (h d) -> p h d", d=HD)
                hi_v = hi_bf16(ps[:, 0:C]).rearrange("p (h d) -> p h d", d=HD)
                for par in (0, 1):
                    dst = vz[:, t, par, :, par * HD:par * HD + HD]
                    nc.vector.tensor_scalar(zt8v, hi_v[:, par::2, :], -0.5, 2.0,
                                            op0=AL.is_gt, op1=AL.mult)
                    nc.vector.tensor_scalar_add(dst, zt8v, -1.0)
            else:
                hv = hi_bf16(ps[:, 0:C]).rearrange("p (h d) -> p h d", d=HD)
                nc.scalar.activation(vz[:, t, 0, :, 0:HD], hv[:, 0::2, :],
                                     AF.Sign, bias=1.0, scale=2.0)
                nc.scalar.activation(vz[:, t, 1, :, HD:P], hv[:, 1::2, :],
                                     AF.Sign, bias=1.0, scale=2.0)

        if dbg:
            dbg_dump(nc, "qkT", qkT[:])
            dbg_dump(nc, "kza", kza[:])
            dbg_dump(nc, "kzb", kzb[:])
            dbg_dump(nc, "vz", vz[:])

        # fc1 weights arrive during attention (free slot of the wbig pool)
        w1T = pwbig.tile([P, CT, HID], FP8, tag="wbig")
        nc.sync.dma_start(w1T[:], w1T_d.rearrange("(k p) o -> p k o", p=P))

        # ---- colsum of v per head (bias for +-1-encoded heads) ----------
        # cb_all[:, p] = sum_m v[m, c] + 1 for c-tile p (c = head*64+d),
        # memset to 1.0 for {0,2}-encoded head halves.  DoubleRow over
        # nt-chunk pairs: 4 matmuls per pair instead of 8.
        cb_all = pc.tile([P, H // 2], F32, tag="cball")
        for p_ in range(H // 2):
            h0in = 2 * p_ in ACT_HEADS
            h1in = 2 * p_ + 1 in ACT_HEADS
            if h0in or h1in:
                csp = pS.tile([P, 1], F32, tag="S", name=f"csp{p_}")
                srcs = ([0] if h0in else []) + ([1] if h1in else [])
                tot = (nt // 2) * len(srcs)
                nmm = 0
                for q in range(nt // 2):
                    for hh in srcs:
                        nc.tensor.matmul(csp[:], lhsT=vz[:, 2 * q:2 * q + 2, hh, p_, :],
                                         rhs=ones2[:], start=(nmm == 0),
                                         stop=(nmm == tot - 1), perf_mode=DR)
                        nmm += 1
                nc.scalar.activation(cb_all[:, p_:p_ + 1], csp[:],
                                     AF.Identity, bias=1.0, scale=1.0)
                if not h0in:
                    nc.vector.memset(cb_all[0:HD, p_:p_ + 1], 1.0)
                if not h1in:
                    nc.vector.memset(cb_all[HD:P, p_:p_ + 1], 1.0)
            else:
                nc.vector.memset(cb_all[:, p_:p_ + 1], 1.0)

        # ---- attention: software-pipelined S(p+1) before O(p) -----------
        soT = pch.tile([P, CT, N], FP8, tag="ch")
        n_pairs = H // 2
        st_tiles = {}

        def alloc_S(p_):
            st0 = pst.tile([P, nt, N], FP8, tag="st", name=f"st{2 * p_}")
            st1 = pst.tile([P, nt, N], FP8, tag="st", name=f"st{2 * p_ + 1}")
            st_tiles[p_] = (st0, st1)

        def emit_S_mt(p_, mt):
            st0, st1 = st_tiles[p_]
            for hh in (0, 1):
                head = 2 * p_ + hh
                st = (st0, st1)[hh]
                kz = (kza, kzb)[hh]
                ps = pS.tile([P, N], F32, tag="S", name=f"sps{head}_{mt}")
                for (n0, nsz) in NCH:
                    # S^T[m,n] = sum_d k^T[d,m] q^T[d,n], K=128 w/ zeros
                    nc.tensor.matmul(
                        ps[:, n0:n0 + nsz],
                        lhsT=kz[:, p_, mt * P:(mt + 1) * P],
                        rhs=qkT[:, p_, n0:n0 + nsz],
                        start=True, stop=True)
                if head in ACT_HEADS:
                    # +-1 encoding: Sign(S-1); S even => never 0
                    nc.scalar.activation(st[:, mt, :], hi_bf16(ps[:]), AF.Sign,
                                         bias=negone[:, 0:1], scale=1.0)
                else:
                    # {0,2} encoding: (S>0)*2; S is small-integer so the
                    # bf16 high halves are exact and read at 2x
                    nc.vector.tensor_scalar(st[:, mt, :], hi_bf16(ps[:]), 0.0, 2.0,
                                            op0=AL.is_gt, op1=AL.mult)

        ot_tiles = {}

        def emit_O_j(p_, j):
            # one psum bank per n-chunk; both heads accumulate into it
            # ([O_h0^T ; 0] + [0 ; O_h1^T]) with full-M DoubleRow matmuls.
            st0, st1 = st_tiles[p_]
            if j == 0:
                ot_tiles[p_] = pO.tile([P, N], F32, tag="oacc",
                                       name=f"ot{p_}")
            ots = ot_tiles[p_]
            nj = nt // 2
            for hh, st in ((0, st0), (1, st1)):
                for (n0, nsz) in NCH:
                    nc.tensor.matmul(
                        ots[:, n0:n0 + nsz],
                        lhsT=vz[:, 2 * j:2 * j + 2, hh, p_, :],
                        rhs=st[:, 2 * j:2 * j + 2, n0:n0 + nsz],
                        start=(j == 0 and hh == 0),
                        stop=(j == nj - 1 and hh == 1), perf_mode=DR)

        def emit_O_tail(p_):
            st_tiles.pop(p_)
            ots = ot_tiles.pop(p_)
            nc.scalar.activation(soT[:, p_, :], hi_bf16(ots[:]), AF.Sign,
                                 bias=cb_all[:, p_:p_ + 1], scale=1.0)

        def dbg_dump_st(p_):
            if not dbg:
                return
            st0, st1 = st_tiles[p_]
            dbg_dump(nc, f"st{2 * p_}", st0[:])
            dbg_dump(nc, f"st{2 * p_ + 1}", st1[:])

        # software pipeline at mt granularity: while pair p_'s S tiles are
        # produced (gated by the binarize evacs), the previous pair's O
        # matmuls are interleaved in the PE stream so the engine never
        # stalls behind a pending evacuation.
        with nc.named_scope("attn"):
            alloc_S(0)
            for mt in range(nt):
                emit_S_mt(0, mt)
            dbg_dump_st(0)
            # fc2 weights arrive during attention (wqkvT's slot is free now)
            w2T = pwbig.tile([P, HT, C], FP8, tag="wbig")
            nc.sync.dma_start(w2T[:], w2T_d.rearrange("(k p) o -> p k o", p=P))
            for p_ in range(1, n_pairs):
                alloc_S(p_)
                for mt in range(nt):
                    emit_S_mt(p_, mt)
                    if mt % 2 == 1:
                        emit_O_j(p_ - 1, mt // 2)
                dbg_dump_st(p_)
                emit_O_tail(p_ - 1)
            for j in range(nt // 2):
                emit_O_j(n_pairs - 1, j)
            emit_O_tail(n_pairs - 1)
        dbg_dump(nc, "cball", cb_all[:])
        dbg_dump(nc, "soT", soT[:])

        # ---- proj + residual + LN2 (per token tile, interleaved) --------
        h2 = ptok.tile([P, nt, C], BF16, tag="tok")
        h2T = pch.tile([P, CT, N], FP8, tag="ch")
        if not cfg.ln2_fast:
            g2r, b2r = opt["g2r"], opt["b2r"]
            h2f = ptok.tile([P, nt, C], F32, tag="h2f")

        def ln2_tail(t):
            # LN2 stats + h2 + transposes for tile t (runs one tile behind
            # the proj matmuls so PE never waits on this serial chain).
            x_t = xt[:, t, :]
            x_h = hi_bf16(x_t)
            nc.vector.bn_stats(bn6[:, 0, :], x_h[:, :C // 2])
            nc.vector.bn_stats(bn6[:, 1, :], x_h[:, C // 2:])
            nc.vector.bn_aggr(mv[:, 2 * t:2 * t + 2], bn6[:])
            rsqrt_cols(r2[:, t:t + 1], mv[:, 2 * t + 1:2 * t + 2], t)
            nc.vector.tensor_scalar_mul(nmu2[:, t:t + 1], mv[:, 2 * t:2 * t + 1],
                                        -1.0)
            nc.vector.tensor_tensor(nmr2[:, t:t + 1], nmu2[:, t:t + 1],
                                    r2[:, t:t + 1], op=AL.mult)
            if cfg.ln2_fast:
                # h2 = (x1 - mu)*r on ScalarE: func(x*scale + bias)
                nc.scalar.activation(h2[:, t, :], hi_bf16(x_t), AF.Identity,
                                     bias=nmr2[:, t:t + 1],
                                     scale=r2[:, t:t + 1])
            else:
                u = h2f[:, t, :]
                nc.vector.tensor_scalar(u, x_t, nmu2[:, t:t + 1], r2[:, t:t + 1],
                                        op0=AL.add, op1=AL.mult)
                nc.vector.tensor_tensor(u, u, g2r[:], op=AL.mult)
                nc.vector.tensor_tensor(h2[:, t, :], u, b2r[:], op=AL.add)


        for t in range(nt):
            ps = pS.tile([P, C], F32, tag="S", name=f"prj{t}")
            for (o0, osz) in _nchunks(C):
                for j in range(CT // 2):
                    nc.tensor.matmul(
                        ps[:, o0:o0 + osz],
                        lhsT=soT[:, 2 * j:2 * j + 2, t * P:(t + 1) * P],
                        rhs=wpT[:, 2 * j:2 * j + 2, o0:o0 + osz],
                        start=(j == 0), stop=(j == CT // 2 - 1), perf_mode=DR)
            x_t = xt[:, t, :]
            # x1 = x + psum * ls1/64  (alpha_p*64 folded into wpT), or the
            # general per-channel path when ls1 is not uniform.
            if uni1:
                nc.scalar.activation(ps[:], hi_bf16(ps[:]), AF.Identity,
                                     bias=0.0, scale=cfg.ls1s)
            else:
                nc.vector.tensor_tensor(ps[:], ps[:], opt["cp1r"][:], op=AL.mult)
            nc.vector.tensor_tensor(x_t, x_t, ps[:], op=AL.add)
            if cfg.has_cp2:
                nc.vector.tensor_tensor(x_t, x_t, opt["cp2r"][:], op=AL.add)
            if t > 0:
                ln2_tail(t - 1)
            if t > 1:
                tr_tile(t - 2, h2, h2T, "tr2_")
        ln2_tail(nt - 1)
        tr_tile(nt - 2, h2, h2T, "tr2_")
        tr_tile(nt - 1, h2, h2T, "tr2_")

        dbg_dump(nc, "x1", xt[:])
        dbg_dump(nc, "h2", h2[:])

        # ---- fc1 -> gelu -> mgT (h-major) -------------------------------
        # w1T carries a1*64; gelu arg = psum/64 + b1.
        mgT = [pst.tile([P, 8, N], FP8, tag="st", name=f"mgT{j}")
               for j in range((HT + 7) // 8)]
        for ht in range(HT):
            ps = pS.tile([P, N], F32, tag="S", name=f"f1_{ht}")
            for (n0, nsz) in NCH:
                for j in range(CT // 2):
                    nc.tensor.matmul(
                        ps[:, n0:n0 + nsz],
                        lhsT=w1T[:, 2 * j:2 * j + 2, ht * P:(ht + 1) * P],
                        rhs=h2T[:, 2 * j:2 * j + 2, n0:n0 + nsz],
                        start=(j == 0), stop=(j == CT // 2 - 1), perf_mode=DR)
            nc.scalar.activation(mgT[ht // 8][:, ht % 8, :], hi_bf16(ps[:]),
                                 GELU_FN, bias=b1s[:, ht:ht + 1],
                                 scale=1.0 / 64.0)

        if dbg:
            for j, mg in enumerate(mgT):
                dbg_dump(nc, f"mgT{j}", mg[:])

        # ---- fc2 + residual -> out --------------------------------------
        for t in range(nt):
            ps = pS.tile([P, C], F32, tag="S", name=f"f2_{t}")
            for (o0, osz) in _nchunks(C):
                for j in range(HT // 2):
                    mg = mgT[j // 4]
                    k0 = (j % 4) * 2
                    nc.tensor.matmul(
                        ps[:, o0:o0 + osz],
                        lhsT=mg[:, k0:k0 + 2, t * P:(t + 1) * P],
                        rhs=w2T[:, 2 * j:2 * j + 2, o0:o0 + osz],
                        start=(j == 0), stop=(j == HT // 2 - 1), perf_mode=DR)
            x_t = xt[:, t, :]
            # out = x1 + psum * ls2/64 (alpha2*64 folded into w2T)
            if uni2:
                nc.scalar.activation(ps[:], hi_bf16(ps[:]), AF.Identity,
                                     bias=0.0, scale=cfg.ls2s)
            else:
                nc.vector.tensor_tensor(ps[:], ps[:], opt["c1r"][:], op=AL.mult)
            nc.vector.tensor_tensor(x_t, x_t, ps[:], op=AL.add)
            if cfg.has_c2:
                nc.vector.tensor_tensor(x_t, x_t, opt["c2r"][:], op=AL.add)
            nc.sync.dma_start(
                out_d.rearrange("(t p) c -> t p c", p=P)[t], x_t)

    nc.compile()
    input_names = ["x", "wqkvT", "wpT", "w1T", "w2T", "b1s"] + list(opt_d.keys())
    if dbg:
        return nc, input_names, dbg_t
    return nc, input_names


# -------------------------------------------------------------------------
# host-side prep + execution
# -------------------------------------------------------------------------

def _sgn(a):
    return np.where(a >= 0, np.float32(1.0), np.float32(-1.0))


def prep_host_inputs(inputs, cfg: Cfg):
    """Returns dict of per-core-common host arrays keyed by dram names."""
    f8 = ml_dtypes.float8_e4m3
    w_qkv = np.asarray(inputs["w_qkv"], np.float32)
    w_proj = np.asarray(inputs["w_proj"], np.float32)
    w_fc1 = np.asarray(inputs["w_fc1"], np.float32)
    w_fc2 = np.asarray(inputs["w_fc2"], np.float32)
    ls1 = np.asarray(inputs["ls1_g"], np.float32)
    ls2 = np.asarray(inputs["ls2_g"], np.float32)
    b_proj = np.asarray(inputs["b_proj"], np.float32)
    b_fc1 = np.asarray(inputs["b_fc1"], np.float32)
    b_fc2 = np.asarray(inputs["b_fc2"], np.float32)

    ap = np.abs(w_proj).mean(axis=1)    # [C] alpha_proj
    a1 = np.abs(w_fc1).mean(axis=1)     # [HID]
    a2 = np.abs(w_fc2).mean(axis=1)     # [C]

    uni1 = cfg.ls1s != 0.0
    uni2 = cfg.ls2s != 0.0
    # alpha*64 folded into the fp8 sign weights (x64 keeps the values in
    # fp8e4m3 normal range; the rel. quantization error ~6% enters the
    # output only through the 1e-5-scaled residual branches).
    wp_scale = (ap * 64.0)[None, :] if uni1 else np.float32(1.0)
    w1_scale = (a1 * 64.0)[:, None]
    w2_scale = (a2 * 64.0)[:, None] if uni2 else np.float32(1.0)

    d = {
        "wqkvT": np.ascontiguousarray(_sgn(w_qkv).T).astype(f8),
        "wpT": np.ascontiguousarray(_sgn(w_proj).T * wp_scale).astype(f8),
        "w1T": np.ascontiguousarray((_sgn(w_fc1) * w1_scale).T).astype(f8),
        "w2T": np.ascontiguousarray((_sgn(w_fc2) * w2_scale).T).astype(f8),
        "b1s": np.ascontiguousarray(b_fc1.reshape(HT, P).T),
    }
    if not uni1:
        d["cp1r"] = np.ascontiguousarray(
            np.broadcast_to(ls1 * ap, (P, C)).copy())
        # w1 always folds a1; gelu scale 1/64 stays correct either way
    if not uni2:
        d["c1r"] = np.ascontiguousarray(
            np.broadcast_to(ls2 * a2, (P, C)).copy())
    if cfg.has_cp2:
        d["cp2r"] = np.ascontiguousarray(np.broadcast_to(ls1 * b_proj, (P, C)).copy())
    if cfg.has_c2:
        d["c2r"] = np.ascontiguousarray(np.broadcast_to(ls2 * b_fc2, (P, C)).copy())
    if not cfg.ln1_fast:
        d["g1r"] = np.ascontiguousarray(
            np.broadcast_to(np.asarray(inputs["ln1_g"], np.float32), (P, C)).copy())
        d["b1r"] = np.ascontiguousarray(
            np.broadcast_to(np.asarray(inputs["ln1_b"], np.float32), (P, C)).copy())
    if not cfg.ln2_fast:
        d["g2r"] = np.ascontiguousarray(
            np.broadcast_to(np.asarray(inputs["ln2_g"], np.float32), (P, C)).copy())
        d["b2r"] = np.ascontiguousarray(
            np.broadcast_to(np.asarray(inputs["ln2_b"], np.float32), (P, C)).copy())
    return d


def make_cfg(inputs, nt=8):
    ln1_g = np.asarray(inputs["ln1_g"], np.float32)
    ln1_b = np.asarray(inputs["ln1_b"], np.float32)
    ln2_g = np.asarray(inputs["ln2_g"], np.float32)
    ln2_b = np.asarray(inputs["ln2_b"], np.float32)
    ls1 = np.asarray(inputs["ls1_g"], np.float32)
    ls2 = np.asarray(inputs["ls2_g"], np.float32)
    b_proj = np.asarray(inputs["b_proj"], np.float32)
    b_fc2 = np.asarray(inputs["b_fc2"], np.float32)
    uni1 = bool(np.all(ls1 == ls1[0]))
    uni2 = bool(np.all(ls2 == ls2[0]))
    return Cfg(
        nt=nt,
        ln1_fast=bool(np.all(ln1_b == 0) and np.all(ln1_g > 0)),
        ln2_fast=bool(np.all(ln2_g == 1) and np.all(ln2_b == 0)),
        has_cp2=bool(np.any(ls1 * b_proj != 0)),
        has_c2=bool(np.any(ls2 * b_fc2 != 0)),
        ls1s=float(ls1[0]) / 64.0 if uni1 else 0.0,
        ls2s=float(ls2[0]) / 64.0 if uni2 else 0.0,
    )


_PROG_CACHE = {}


def get_program(cfg: Cfg):
    key = cfg
    if key not in _PROG_CACHE:
        _PROG_CACHE[key] = build_program(cfg)
    return _PROG_CACHE[key]


def kernel(**inputs):
    from concourse.bass_utils import run_bass_kernel_spmd

    x = np.asarray(inputs["x"], np.float32)
    assert x.shape == (B, 1024, C), x.shape
    cfg = make_cfg(inputs, nt=1024 // P)
    nc, _names = get_program(cfg)
    common = prep_host_inputs(inputs, cfg)

    in_maps = []
    for b in range(B):
        m = dict(common)
        m["x"] = np.ascontiguousarray(x[b])
        in_maps.append(m)

    res = run_bass_kernel_spmd(nc, in_maps, core_ids=list(range(N_CORES)))
    out = np.stack([res.results[b]["out"] for b in range(B)], axis=0)
    return out.astype(np.float32)
